# revision 1
# baseline (speedup 1.0000x reference)
"""DILATE loss (soft-DTW fwd + grad, gamma=0.01 ~ hard-min) on 8 TRN2 cores.

Batch-parallel: 8 samples/core. Per core, the 64 (sample, col-block) DP scans
run as a skewed wavefront: 4 col-blocks of 64 columns, block q on SBUF
quadrant q (lanes 32q+0..7). Slot t of block q holds DP row i = t - q in a
65-float record [chain | 64 cols]; tensor_tensor_scan computes each row's
min-plus recurrence in one instruction, with the cross-block chain value
injected as scan element 0 via quadrant-aligned copies. The soft-DTW gradient
is the hard argmin-mask linear recurrence run as a reversed scan; masks are
equality-derived in batched chunks and bounced through DRAM.
"""
import numpy as np
import ml_dtypes

bf16 = ml_dtypes.bfloat16
f32 = np.float32

ALPHA = 0.5
BIG = 1e8
B, N = 64, 256
Q, C = 4, 65
S, SE = 260, 262
NCORES = 8
SPC = B // NCORES
MCH = 8    # mask-phase chunk (slots)
WCH = 8    # backward mask window stride (slots); window covers WCH+1 slots

_cache = {}


def _build(repeat=1, phases="dfmbx", fwdops="cms"):
    import concourse.bacc as bacc
    import concourse.tile as tile
    import concourse.mybir as mybir
    from contextlib import ExitStack

    dt = mybir.dt
    Alu = mybir.AluOpType

    nc = bacc.Bacc("TRN2", target_bir_lowering=False, debug=False)
    dT_d = nc.dram_tensor("dT", [128, S], dt.float32, kind="ExternalInput").ap()
    dO_d = nc.dram_tensor("dO", [128, 64], dt.float32, kind="ExternalInput").ap()
    mx_d = nc.dram_tensor("mx", [128, S * C], dt.bfloat16, kind="ExternalInput").ap()
    ps_d = nc.dram_tensor("ps", [8, 1], dt.float32, kind="ExternalOutput").ap()
    pt_d = nc.dram_tensor("pt", [128, 1], dt.float32, kind="ExternalOutput").ap()
    mU_d = nc.dram_tensor("mU_s", [128, SE * C], dt.bfloat16).ap()
    mD_d = nc.dram_tensor("mD_s", [128, SE * C], dt.bfloat16).ap()
    mL_d = nc.dram_tensor("mL_s", [128, SE * C], dt.bfloat16).ap()

    with tile.TileContext(nc) as tc:
        with ExitStack() as ctx:
            big = ctx.enter_context(tc.tile_pool(name="big", bufs=1))
            st_pool = ctx.enter_context(tc.tile_pool(name="stage", bufs=2))
            win_pool = ctx.enter_context(tc.tile_pool(name="win", bufs=2))
            sc_pool = ctx.enter_context(tc.tile_pool(name="scr", bufs=2))

            h = big.tile([128, S * C], dt.float32, tag="h")
            d = big.tile([128, S * C], dt.bfloat16, tag="d")
            E = big.tile([128, SE * C], dt.float32, tag="E")
            dT = big.tile([128, S], dt.float32, tag="dT")
            dO = big.tile([128, 64], dt.float32, tag="dO")
            c0 = big.tile([128, C], dt.float32, tag="c0")
            c1 = big.tile([128, C], dt.float32, tag="c1")
            G0 = big.tile([128, 66], dt.float32, tag="G0")
            G1t = big.tile([128, 66], dt.float32, tag="G1t")
            S0 = big.tile([128, 66], dt.float32, tag="S0")
            S1t = big.tile([128, 66], dt.float32, tag="S1t")
            zb = big.tile([128, 2 * C], dt.bfloat16, tag="zb")
            pt_t = big.tile([128, 1], dt.float32, tag="pt_t")
            c_tiles = [c0, c1]
            G_tiles = [G0, G1t]
            S_tiles = [S0, S1t]

            # inputs
            nc.sync.dma_start(out=dT[:], in_=dT_d[:])
            nc.sync.dma_start(out=dO[:], in_=dO_d[:])
            for _rep in range(repeat):
                # E zero on gpsimd (runs concurrent with fwd on DVE)
                nc.gpsimd.memset(E[:], 0.0)
                nc.gpsimd.memset(zb[:], 0.0)

                # D build: d[p, t*C+1+jl] = (dT[p,t]-dO[p,jl])^2  (bf16)
                if "d" not in phases:
                    continue
                nc.vector.memset(d[:], 0.0)
                DCH = 33
                for k0 in range(0, S, DCH):
                    k1 = min(k0 + DCH, S)
                    d3 = d[:].rearrange("p (s c) -> p s c", c=C)[:, k0:k1, 1:]
                    nc.vector.tensor_tensor(
                        d3, dT[:, k0:k1].unsqueeze(2).broadcast_to([128, k1 - k0, 64]),
                        dO[:].unsqueeze(1).broadcast_to([128, k1 - k0, 64]), Alu.subtract)
                    nc.vector.tensor_tensor(d3, d3, d3, Alu.mult)

                # fwd prefills
                for q in range(Q):
                    nc.vector.memset(h[32 * q:32 * q + 32, q * C:(q + 1) * C], BIG)
                nc.vector.memset(h[0:8, 0:1], 0.0)
                for ct in c_tiles:
                    nc.vector.memset(ct[0:32, 0:1], BIG)
                for gt in G_tiles:
                    nc.vector.memset(gt[:, 0:1], 0.0)
                    nc.vector.memset(gt[96:128, 65:66], 0.0)

                # ---------------- forward ----------------
                if "f" not in phases:
                    continue
                def prange(qlo, qhi):
                    P0, P1 = 32 * qlo, 32 * qhi + 32
                    cnt = P1 - P0
                    if not (cnt <= 32 or P0 == 0 or (P0 == 64 and cnt <= 64)):
                        P0 = 0
                    return P0, P1

                for t in range(1, S):
                    qlo, qhi = max(0, t - 256), min(3, t - 1)
                    P0, P1 = prange(qlo, qhi)
                    ct = c_tiles[t % 2]
                    if "c" in fwdops:
                        for q in range(max(1, qlo), qhi + 1):
                            nc.gpsimd.tensor_copy(
                                ct[32 * q:32 * q + 32, 0:1],
                                h[32 * (q - 1):32 * q, (t - 1) * C + 64:(t - 1) * C + 65])
                    if "m" in fwdops:
                        nc.vector.tensor_tensor(
                            ct[P0:P1, 1:65],
                            h[P0:P1, (t - 1) * C + 1:(t - 1) * C + 65],
                            h[P0:P1, (t - 1) * C:(t - 1) * C + 64], Alu.min)
                    if "s" in fwdops:
                        # state = min(c'_j, state) + d_j  (c' excludes d; chain in c'[0])
                        nc.vector.tensor_tensor_scan(
                            h[P0:P1, t * C:t * C + 65],
                            ct[P0:P1, 0:65],
                            d[P0:P1, t * C:t * C + 65], float(BIG), Alu.min, Alu.add)

                # loss_shape partials
                nc.sync.dma_start(out=ps_d[:], in_=h[96:104, 259 * C + 64:259 * C + 65])

                # ---------------- mask phase ----------------
                if "m" not in phases:
                    continue
                for s0 in range(1, S, MCH):
                    s1 = min(s0 + MCH, S)
                    ns = s1 - s0
                    cX = sc_pool.tile([128, MCH * C], dt.float32, tag="cX")
                    mu = st_pool.tile([128, MCH * C], dt.bfloat16, tag="mu")
                    md = st_pool.tile([128, MCH * C], dt.bfloat16, tag="md")
                    ml = st_pool.tile([128, MCH * C], dt.bfloat16, tag="ml")
                    hv = h[:].rearrange("p (s c) -> p s c", c=C)
                    dv = d[:].rearrange("p (s c) -> p s c", c=C)
                    cXv = cX[:].rearrange("p (s c) -> p s c", c=C)[:, 0:ns, :]
                    for m_t, hoff in ((mu, hv[:, s0 - 1:s1 - 1, 1:]),
                                      (md, hv[:, s0 - 1:s1 - 1, 0:64]),
                                      (ml, hv[:, s0:s1, 0:64])):
                        nc.vector.tensor_tensor(cXv[:, :, 1:], dv[:, s0:s1, 1:], hoff, Alu.add)
                        mv = m_t[:].rearrange("p (s c) -> p s c", c=C)[:, 0:ns, :]
                        nc.vector.tensor_tensor(mv[:, :, 1:], hv[:, s0:s1, 1:],
                                                cXv[:, :, 1:], Alu.is_equal)
                    # margins on md, ml
                    for m_t in (md, ml):
                        mv = m_t[:].rearrange("p (s c) -> p s c", c=C)[:, 0:ns, :]
                        for q in (0, 1, 2):
                            nc.gpsimd.tensor_copy(
                                mv[32 * q:32 * q + 32, :, 0:1],
                                mv[32 * (q + 1):32 * (q + 1) + 32, :, 1:2])
                        nc.gpsimd.memset(mv[96:128, :, 0:1], 0.0)
                    for m_t, m_dram in ((mu, mU_d), (md, mD_d), (ml, mL_d)):
                        nc.sync.dma_start(out=m_dram[0:104, s0 * C:s1 * C],
                                          in_=m_t[0:104, 0:ns * C])
                # zero-fill DRAM mask slots 260..261
                for m_dram in (mU_d, mD_d, mL_d):
                    nc.sync.dma_start(out=m_dram[0:104, 260 * C:262 * C], in_=zb[0:104, :])

                # X DMA-in over d (all mask-phase reads of d are done)
                nc.sync.dma_start(out=d[:], in_=mx_d[:])

                # ---------------- backward ----------------
                if "b" not in phases:
                    continue
                def win_load(k):
                    w0 = k * WCH
                    nsl = min(WCH + 2, SE - w0)
                    tiles = {}
                    for name, m_dram in (("u", mU_d), ("d", mD_d), ("l", mL_d)):
                        w = win_pool.tile([128, (WCH + 2) * C], dt.bfloat16, tag="w" + name)
                        nc.sync.dma_start(out=w[0:104, 0:nsl * C],
                                          in_=m_dram[0:104, w0 * C:(w0 + nsl) * C])
                        tiles[name] = w
                    return tiles

                cur_k = (S - 1) // WCH
                wins = {cur_k: win_load(cur_k)}
                if cur_k - 1 >= 0:
                    wins[cur_k - 1] = win_load(cur_k - 1)
                for t in range(S - 1, 0, -1):
                    k = t // WCH
                    if k != cur_k:
                        cur_k = k
                        wins.pop(k + 2, None)
                        if k - 1 >= 0 and (k - 1) not in wins:
                            wins[k - 1] = win_load(k - 1)
                    W = wins[k]
                    lo = (t - k * WCH) * C
                    qlo, qhi = max(0, t - 256), min(3, t - 1)
                    P0, P1 = prange(qlo, qhi)
                    G = G_tiles[t % 2]
                    Sc = S_tiles[t % 2]
                    for q in (2, 1, 0):
                        nc.vector.tensor_copy(
                            G[32 * q:32 * q + 32, 65:66],
                            E[32 * (q + 1):32 * (q + 2), (t + 1) * C + 1:(t + 1) * C + 2])
                    nc.vector.tensor_tensor(
                        G[P0:P1, 1:65], E[P0:P1, (t + 1) * C + 1:(t + 1) * C + 65],
                        W["u"][P0:P1, lo + C + 1:lo + C + 65], Alu.mult)
                    nc.vector.tensor_tensor(
                        Sc[P0:P1, 1:65], E[P0:P1, (t + 1) * C + 2:(t + 1) * C + 66],
                        W["d"][P0:P1, lo + C + 2:lo + C + 66], Alu.mult)
                    nc.vector.tensor_tensor(G[P0:P1, 1:65], G[P0:P1, 1:65],
                                            Sc[P0:P1, 1:65], Alu.add)
                    if t == S - 1:
                        nc.vector.memset(G[96:128, 64:65], 1.0)
                    nc.vector.tensor_tensor_scan(
                        E[P0:P1, t * C:t * C + 66][:, ::-1],
                        W["l"][P0:P1, lo + 1:lo + 67][:, ::-1],
                        G[P0:P1, 0:66][:, ::-1], 0.0, Alu.mult, Alu.add)

                # ---------------- omega reduction ----------------
                if "x" not in phases:
                    continue
                nc.vector.tensor_tensor(E[0:104, 0:S * C], E[0:104, 0:S * C],
                                        d[0:104, 0:S * C], Alu.mult)
                nc.vector.tensor_tensor(E[0:104, 0:S * C], E[0:104, 0:S * C],
                                        d[0:104, 0:S * C], Alu.mult)
                nc.vector.tensor_reduce(
                    pt_t[0:104, 0:1],
                    E[0:104, 0:S * C].rearrange("p (s c) -> p s c", c=C),
                    mybir.AxisListType.XY, Alu.add)
                nc.sync.dma_start(out=pt_d[:], in_=pt_t[:])

    nc.compile()
    return nc


def _host_inputs(y_true, y_pred):
    """Per-core input dict list. y_true/y_pred: [B, N] f32."""
    in_maps = []
    rows = np.arange(S)  # slot t
    for core in range(NCORES):
        dT = np.zeros((128, S), f32)
        dO = np.zeros((128, 64), f32)
        mx = np.zeros((128, S, C), bf16)
        for q in range(Q):
            i = rows - q  # row index per slot
            valid = (i >= 1) & (i <= N)
            for s in range(SPC):
                b = core * SPC + s
                p = 32 * q + s
                dT[p, valid] = y_true[b, i[valid] - 1]
                dO[p, :] = y_pred[b, q * 64:(q + 1) * 64]
                m = np.arange(1, 65)[None, :]
                X = (i[:, None] - (q * 64 + m)).astype(f32)
                X[~valid, :] = 0.0
                mx[p, :, 1:] = X.astype(bf16)
        in_maps.append({"dT": dT, "dO": dO, "mx": mx.reshape(128, S * C)})
    return in_maps


def kernel(y_pred, y_true):
    yp = np.asarray(y_pred, dtype=f32).reshape(B, N)
    yt = np.asarray(y_true, dtype=f32).reshape(B, N)
    if "nc" not in _cache:
        _cache["nc"] = _build()
    nc = _cache["nc"]
    from concourse.bass_utils import run_bass_kernel_spmd
    in_maps = _host_inputs(yt, yp)
    res = run_bass_kernel_spmd(nc, in_maps, core_ids=list(range(NCORES)))
    shape_vals = []
    temp_sum = 0.0
    for core in range(NCORES):
        r = res.results[core]
        shape_vals.append(r["ps"][:, 0])
        pt = r["pt"][:, 0]
        for q in range(Q):
            for s in range(SPC):
                temp_sum += float(pt[32 * q + s])
    loss_shape = float(np.mean(np.concatenate(shape_vals)))
    loss_temporal = temp_sum / B / (N * N)
    loss = ALPHA * loss_shape + (1.0 - ALPHA) * loss_temporal
    return np.array(loss, dtype=f32)



# revision 5
# speedup vs baseline: 5.9493x; 5.9493x over previous
"""DILATE loss (soft-DTW fwd + grad, gamma=0.01 ~ hard-min) on 8 TRN2 cores.

Batch-parallel: 8 samples/core. Per core, the 64 (sample, col-block) DP scans
run as a skewed wavefront: 4 col-blocks of 64 columns, block q on SBUF
quadrant q (lanes 32q+0..7). Slot t of block q holds DP row i = t - q in a
65-float record [chain | 64 cols]; tensor_tensor_scan computes each row's
min-plus recurrence in one instruction, with the cross-block chain value
injected as scan element 0 via quadrant-aligned copies. The soft-DTW gradient
is the hard argmin-mask linear recurrence run as a reversed scan; masks are
equality-derived in batched chunks and bounced through DRAM.

Host<->device traffic is minimal: inputs are the raw per-core y_true/y_pred
rows ([8,256] f32 each); the skewed dT/dO layout and the omega=(i-j)^2
penalty matrix are built on-device (iota + quadrant copies). The jitted
shard_map dispatcher is built once and cached — per-call work is a couple of
small numpy reshapes plus one jit invocation.
"""
import numpy as np

f32 = np.float32

ALPHA = 0.5
BIG = 1e8
B, N = 64, 256
Q, C = 4, 65
S, SE = 260, 262
NCORES = 8
SPC = B // NCORES
MCH = 8    # mask-phase chunk (slots)
WCH = 8    # backward mask window stride (slots); window covers WCH+1 slots

_cache = {}


def _build():
    import concourse.bacc as bacc
    import concourse.tile as tile
    import concourse.mybir as mybir
    from contextlib import ExitStack

    dt = mybir.dt
    Alu = mybir.AluOpType

    nc = bacc.Bacc("TRN2", target_bir_lowering=False, debug=False)
    yt_d = nc.dram_tensor("yt", [SPC, N], dt.float32, kind="ExternalInput").ap()
    yp_d = nc.dram_tensor("yp", [SPC, N], dt.float32, kind="ExternalInput").ap()
    ps_d = nc.dram_tensor("ps", [8, 1], dt.float32, kind="ExternalOutput").ap()
    pt_d = nc.dram_tensor("pt", [128, 1], dt.float32, kind="ExternalOutput").ap()
    mU_d = nc.dram_tensor("mU_s", [128, SE * C], dt.bfloat16).ap()
    mD_d = nc.dram_tensor("mD_s", [128, SE * C], dt.bfloat16).ap()
    mL_d = nc.dram_tensor("mL_s", [128, SE * C], dt.bfloat16).ap()

    with tile.TileContext(nc) as tc:
        with ExitStack() as ctx:
            big = ctx.enter_context(tc.tile_pool(name="big", bufs=1))
            st_pool = ctx.enter_context(tc.tile_pool(name="stage", bufs=2))
            win_pool = ctx.enter_context(tc.tile_pool(name="win", bufs=2))
            sc_pool = ctx.enter_context(tc.tile_pool(name="scr", bufs=2))

            h = big.tile([128, S * C], dt.float32, tag="h")
            d = big.tile([128, S * C], dt.bfloat16, tag="d")
            E = big.tile([128, SE * C], dt.float32, tag="E")
            dT = big.tile([128, S], dt.float32, tag="dT")
            dO = big.tile([128, 64], dt.float32, tag="dO")
            trow = big.tile([128, S], dt.float32, tag="trow")
            nrow = big.tile([128, 64], dt.float32, tag="nrow")
            q65 = big.tile([128, 1], dt.float32, tag="q65")
            c0 = big.tile([128, C], dt.float32, tag="c0")
            c1 = big.tile([128, C], dt.float32, tag="c1")
            G0 = big.tile([128, 66], dt.float32, tag="G0")
            G1t = big.tile([128, 66], dt.float32, tag="G1t")
            S0 = big.tile([128, 66], dt.float32, tag="S0")
            S1t = big.tile([128, 66], dt.float32, tag="S1t")
            zb = big.tile([128, 2 * C], dt.bfloat16, tag="zb")
            pt_t = big.tile([128, 1], dt.float32, tag="pt_t")
            c_tiles = [c0, c1]
            G_tiles = [G0, G1t]
            S_tiles = [S0, S1t]

            # inputs: raw rows DMA'd straight into the skewed layout
            nc.vector.memset(dT[:], 0.0)
            nc.vector.memset(dO[:], 0.0)
            for q in range(Q):
                # dT[32q+s, q+1 : q+1+N] = y_true[s, :]; dO[32q+s, :] = y_pred[s, 64q:64q+64]
                nc.sync.dma_start(out=dT[32 * q:32 * q + SPC, q + 1:q + 1 + N], in_=yt_d[:])
                nc.sync.dma_start(out=dO[32 * q:32 * q + SPC, :], in_=yp_d[:, q * 64:(q + 1) * 64])

            # omega ingredients: trow[p,t]=t ; nrow[p,m]=(m+1)+65*q(p)
            nc.gpsimd.iota(trow[:], [[1, S]], base=0, channel_multiplier=0,
                           allow_small_or_imprecise_dtypes=True)
            nc.gpsimd.iota(nrow[:], [[1, 64]], base=1, channel_multiplier=0,
                           allow_small_or_imprecise_dtypes=True)
            for q in range(Q):
                nc.gpsimd.memset(q65[32 * q:32 * q + 32, 0:1], float(65 * q))
            nc.vector.tensor_tensor(nrow[:], nrow[:],
                                    q65[:].broadcast_to([128, 64]), Alu.add)

            # E zero on gpsimd (runs concurrent with fwd on DVE)
            nc.gpsimd.memset(E[:], 0.0)
            nc.gpsimd.memset(zb[:], 0.0)

            # D build: d[p, t*C+1+jl] = (dT[p,t]-dO[p,jl])^2  (bf16)
            nc.vector.memset(d[:], 0.0)
            DCH = 33
            for k0 in range(0, S, DCH):
                k1 = min(k0 + DCH, S)
                d3 = d[:].rearrange("p (s c) -> p s c", c=C)[:, k0:k1, 1:]
                nc.vector.tensor_tensor(
                    d3, dT[:, k0:k1].unsqueeze(2).broadcast_to([128, k1 - k0, 64]),
                    dO[:].unsqueeze(1).broadcast_to([128, k1 - k0, 64]), Alu.subtract)
                nc.vector.tensor_tensor(d3, d3, d3, Alu.mult)

            # fwd prefills
            for q in range(Q):
                nc.vector.memset(h[32 * q:32 * q + 32, q * C:(q + 1) * C], BIG)
            nc.vector.memset(h[0:8, 0:1], 0.0)
            for ct in c_tiles:
                nc.vector.memset(ct[0:32, 0:1], BIG)
            for gt in G_tiles:
                nc.vector.memset(gt[:, 0:1], 0.0)
                nc.vector.memset(gt[96:128, 65:66], 0.0)

            # ---------------- forward ----------------
            def prange(qlo, qhi):
                P0, P1 = 32 * qlo, 32 * qhi + 32
                cnt = P1 - P0
                if not (cnt <= 32 or P0 == 0 or (P0 == 64 and cnt <= 64)):
                    P0 = 0
                return P0, P1

            for t in range(1, S):
                qlo, qhi = max(0, t - 256), min(3, t - 1)
                P0, P1 = prange(qlo, qhi)
                ct = c_tiles[t % 2]
                for q in range(max(1, qlo), qhi + 1):
                    nc.gpsimd.tensor_copy(
                        ct[32 * q:32 * q + 32, 0:1],
                        h[32 * (q - 1):32 * q, (t - 1) * C + 64:(t - 1) * C + 65])
                nc.vector.tensor_tensor(
                    ct[P0:P1, 1:65],
                    h[P0:P1, (t - 1) * C + 1:(t - 1) * C + 65],
                    h[P0:P1, (t - 1) * C:(t - 1) * C + 64], Alu.min)
                # state = min(c'_j, state) + d_j  (c' excludes d; chain in c'[0])
                nc.vector.tensor_tensor_scan(
                    h[P0:P1, t * C:t * C + 65],
                    ct[P0:P1, 0:65],
                    d[P0:P1, t * C:t * C + 65], float(BIG), Alu.min, Alu.add)

            # loss_shape partials
            nc.sync.dma_start(out=ps_d[:], in_=h[96:104, 259 * C + 64:259 * C + 65])

            # ---------------- mask phase ----------------
            for s0 in range(1, S, MCH):
                s1 = min(s0 + MCH, S)
                ns = s1 - s0
                cX = sc_pool.tile([128, MCH * C], dt.float32, tag="cX")
                mu = st_pool.tile([128, MCH * C], dt.bfloat16, tag="mu")
                md = st_pool.tile([128, MCH * C], dt.bfloat16, tag="md")
                ml = st_pool.tile([128, MCH * C], dt.bfloat16, tag="ml")
                hv = h[:].rearrange("p (s c) -> p s c", c=C)
                dv = d[:].rearrange("p (s c) -> p s c", c=C)
                cXv = cX[:].rearrange("p (s c) -> p s c", c=C)[:, 0:ns, :]
                for m_t, hoff in ((mu, hv[:, s0 - 1:s1 - 1, 1:]),
                                  (md, hv[:, s0 - 1:s1 - 1, 0:64]),
                                  (ml, hv[:, s0:s1, 0:64])):
                    nc.vector.tensor_tensor(cXv[:, :, 1:], dv[:, s0:s1, 1:], hoff, Alu.add)
                    mv = m_t[:].rearrange("p (s c) -> p s c", c=C)[:, 0:ns, :]
                    nc.vector.tensor_tensor(mv[:, :, 1:], hv[:, s0:s1, 1:],
                                            cXv[:, :, 1:], Alu.is_equal)
                # margins on md, ml
                for m_t in (md, ml):
                    mv = m_t[:].rearrange("p (s c) -> p s c", c=C)[:, 0:ns, :]
                    for q in (0, 1, 2):
                        nc.gpsimd.tensor_copy(
                            mv[32 * q:32 * q + 32, :, 0:1],
                            mv[32 * (q + 1):32 * (q + 1) + 32, :, 1:2])
                    nc.gpsimd.memset(mv[96:128, :, 0:1], 0.0)
                for m_t, m_dram in ((mu, mU_d), (md, mD_d), (ml, mL_d)):
                    nc.sync.dma_start(out=m_dram[0:104, s0 * C:s1 * C],
                                      in_=m_t[0:104, 0:ns * C])
            # zero-fill DRAM mask slots 260..261
            for m_dram in (mU_d, mD_d, mL_d):
                nc.sync.dma_start(out=m_dram[0:104, 260 * C:262 * C], in_=zb[0:104, :])

            # omega build over d (all mask-phase reads of d are done):
            # d[p, t*C+1+ml] = (t - (65q + ml+1)) = i - j  for the cell this
            # slot holds; squared later via the double multiply in reduction.
            for k0 in range(0, S, DCH):
                k1 = min(k0 + DCH, S)
                d3 = d[:].rearrange("p (s c) -> p s c", c=C)[:, k0:k1, 1:]
                nc.vector.tensor_tensor(
                    d3, trow[:, k0:k1].unsqueeze(2).broadcast_to([128, k1 - k0, 64]),
                    nrow[:].unsqueeze(1).broadcast_to([128, k1 - k0, 64]), Alu.subtract)
            # zero invalid slots (t outside [q+1, q+256]) and record col 0
            dv = d[:].rearrange("p (s c) -> p s c", c=C)
            nc.vector.memset(dv[:, :, 0:1], 0.0)
            for q in range(Q):
                nc.vector.memset(d[32 * q:32 * q + 32, 0:(q + 1) * C], 0.0)
                if 257 + q < S:
                    nc.vector.memset(d[32 * q:32 * q + 32, (257 + q) * C:S * C], 0.0)

            # ---------------- backward ----------------
            def win_load(k):
                w0 = k * WCH
                nsl = min(WCH + 2, SE - w0)
                tiles = {}
                for name, m_dram in (("u", mU_d), ("d", mD_d), ("l", mL_d)):
                    w = win_pool.tile([128, (WCH + 2) * C], dt.bfloat16, tag="w" + name)
                    nc.sync.dma_start(out=w[0:104, 0:nsl * C],
                                      in_=m_dram[0:104, w0 * C:(w0 + nsl) * C])
                    tiles[name] = w
                return tiles

            cur_k = (S - 1) // WCH
            wins = {cur_k: win_load(cur_k)}
            if cur_k - 1 >= 0:
                wins[cur_k - 1] = win_load(cur_k - 1)
            for t in range(S - 1, 0, -1):
                k = t // WCH
                if k != cur_k:
                    cur_k = k
                    wins.pop(k + 2, None)
                    if k - 1 >= 0 and (k - 1) not in wins:
                        wins[k - 1] = win_load(k - 1)
                W = wins[k]
                lo = (t - k * WCH) * C
                qlo, qhi = max(0, t - 256), min(3, t - 1)
                P0, P1 = prange(qlo, qhi)
                G = G_tiles[t % 2]
                Sc = S_tiles[t % 2]
                for q in (2, 1, 0):
                    nc.vector.tensor_copy(
                        G[32 * q:32 * q + 32, 65:66],
                        E[32 * (q + 1):32 * (q + 2), (t + 1) * C + 1:(t + 1) * C + 2])
                nc.vector.tensor_tensor(
                    G[P0:P1, 1:65], E[P0:P1, (t + 1) * C + 1:(t + 1) * C + 65],
                    W["u"][P0:P1, lo + C + 1:lo + C + 65], Alu.mult)
                nc.vector.tensor_tensor(
                    Sc[P0:P1, 1:65], E[P0:P1, (t + 1) * C + 2:(t + 1) * C + 66],
                    W["d"][P0:P1, lo + C + 2:lo + C + 66], Alu.mult)
                nc.vector.tensor_tensor(G[P0:P1, 1:65], G[P0:P1, 1:65],
                                        Sc[P0:P1, 1:65], Alu.add)
                if t == S - 1:
                    nc.vector.memset(G[96:128, 64:65], 1.0)
                nc.vector.tensor_tensor_scan(
                    E[P0:P1, t * C:t * C + 66][:, ::-1],
                    W["l"][P0:P1, lo + 1:lo + 67][:, ::-1],
                    G[P0:P1, 0:66][:, ::-1], 0.0, Alu.mult, Alu.add)

            # ---------------- omega reduction ----------------
            nc.vector.tensor_tensor(E[0:104, 0:S * C], E[0:104, 0:S * C],
                                    d[0:104, 0:S * C], Alu.mult)
            nc.vector.tensor_tensor(E[0:104, 0:S * C], E[0:104, 0:S * C],
                                    d[0:104, 0:S * C], Alu.mult)
            nc.vector.tensor_reduce(
                pt_t[0:104, 0:1],
                E[0:104, 0:S * C].rearrange("p (s c) -> p s c", c=C),
                mybir.AxisListType.XY, Alu.add)
            nc.sync.dma_start(out=pt_d[:], in_=pt_t[:])

    nc.compile()
    return nc


def _get_runner():
    """Build nc + the jitted shard_map dispatcher once; cache both."""
    import jax
    from jax.sharding import Mesh, PartitionSpec
    import warnings
    with warnings.catch_warnings():
        warnings.simplefilter("ignore")
        from jax.experimental.shard_map import shard_map
    from concourse import mybir
    from concourse.bass2jax import (_bass_exec_p, partition_id_tensor,
                                    install_neuronx_cc_hook)

    nc = _build()
    install_neuronx_cc_hook()

    partition_name = nc.partition_id_tensor.name if nc.partition_id_tensor else None
    in_names, out_names, out_avals, zero_shapes = [], [], [], []
    for alloc in nc.m.functions[0].allocations:
        if not isinstance(alloc, mybir.MemoryLocationSet):
            continue
        name = alloc.memorylocations[0].name
        if alloc.kind == "ExternalInput":
            if name != partition_name:
                in_names.append(name)
        elif alloc.kind == "ExternalOutput":
            out_names.append(name)
            shape = tuple(alloc.tensor_shape)
            dtype = mybir.dt.np(alloc.dtype)
            out_avals.append(jax.core.ShapedArray(shape, dtype))
            zero_shapes.append((shape, dtype))
    n_params = len(in_names)
    n_outs = len(out_names)
    in_names_all = list(in_names) + list(out_names)
    if partition_name is not None:
        in_names_all.append(partition_name)
    donate = tuple(range(n_params, n_params + n_outs))

    dbg_name = nc.dbg_addr.name if nc.dbg_addr is not None else None
    assert dbg_name is None or dbg_name in in_names

    def _body(*args):
        operands = list(args)
        if partition_name is not None:
            operands.append(partition_id_tensor())
        outs = _bass_exec_p.bind(
            *operands, out_avals=tuple(out_avals), in_names=tuple(in_names_all),
            out_names=tuple(out_names), lowering_input_output_aliases=(),
            sim_require_finite=True, sim_require_nnan=True, nc=nc)
        return tuple(outs)

    devices = jax.devices()[:NCORES]
    mesh = Mesh(np.asarray(devices), ("core",))
    in_specs = (PartitionSpec("core"),) * (n_params + n_outs)
    out_specs = (PartitionSpec("core"),) * n_outs
    fn = jax.jit(shard_map(_body, mesh=mesh, in_specs=in_specs,
                           out_specs=out_specs, check_rep=False),
                 donate_argnums=donate, keep_unused=True)
    return {"fn": fn, "in_names": in_names, "out_names": out_names,
            "zero_shapes": zero_shapes}


def kernel(y_pred, y_true):
    yp = np.ascontiguousarray(np.asarray(y_pred, dtype=f32).reshape(B, N))
    yt = np.ascontiguousarray(np.asarray(y_true, dtype=f32).reshape(B, N))
    if "runner" not in _cache:
        _cache["runner"] = _get_runner()
    r = _cache["runner"]

    # concat per-core inputs along axis 0: core c gets rows [c*SPC, (c+1)*SPC)
    feed = {"yt": yt, "yp": yp}
    concat_in = [feed[name] for name in r["in_names"]]
    concat_zeros = [np.zeros((NCORES * s[0], *s[1:]), dt)
                    for s, dt in r["zero_shapes"]]
    out = r["fn"](*concat_in, *concat_zeros)

    outs = {name: np.asarray(out[i]) for i, name in enumerate(r["out_names"])}
    ps = outs["ps"].reshape(NCORES, 8)          # per-core loss_shape partials
    pt = outs["pt"].reshape(NCORES, Q, 32)      # per-lane omega partial sums

    loss_shape = float(np.mean(ps))
    temp_sum = float(np.sum(pt[:, :, :SPC]))
    loss_temporal = temp_sum / B / (N * N)
    loss = ALPHA * loss_shape + (1.0 - ALPHA) * loss_temporal
    return np.array(loss, dtype=f32)


# revision 8
# speedup vs baseline: 11.4167x; 1.9190x over previous
"""DILATE loss (soft-DTW fwd + grad, gamma=0.01 ~ hard-min) on 8 TRN2 cores.

Batch-parallel: 8 samples/core. Per core, the 64 (sample, col-block) DP scans
run as a skewed wavefront: 4 col-blocks of 64 columns, block q on SBUF
quadrant q (lanes 32q+0..7). Slot t of block q holds DP row i = t - q in a
65-float record [chain | 64 cols]; tensor_tensor_scan computes each row's
min-plus recurrence in one instruction, with the cross-block chain value
injected as scan element 0 via quadrant-aligned copies. The soft-DTW gradient
is the hard argmin-mask linear recurrence run as a reversed scan; masks are
equality-derived in batched chunks and bounced through DRAM.

Host<->device traffic is minimal: inputs are the raw per-core y_true/y_pred
rows ([8,256] f32 each); the skewed dT/dO layout and the omega=(i-j)^2
penalty matrix are built on-device (iota + quadrant copies). The jitted
shard_map dispatcher is built once and cached — per-call work is a couple of
small numpy reshapes plus one jit invocation.
"""
import numpy as np

f32 = np.float32

ALPHA = 0.5
BIG = 1e8
B, N = 64, 256
Q, C = 4, 65
S, SE = 260, 262
NCORES = 8
SPC = B // NCORES
MCH = 8    # mask-phase chunk (slots)
WCH = 8    # backward mask window stride (slots); window covers WCH+1 slots

_cache = {}


def _build():
    import concourse.bacc as bacc
    import concourse.tile as tile
    import concourse.mybir as mybir
    from contextlib import ExitStack

    dt = mybir.dt
    Alu = mybir.AluOpType

    nc = bacc.Bacc("TRN2", target_bir_lowering=False, debug=False)
    yt_d = nc.dram_tensor("yt", [SPC, N], dt.float32, kind="ExternalInput").ap()
    yp_d = nc.dram_tensor("yp", [SPC, N], dt.float32, kind="ExternalInput").ap()
    ps_d = nc.dram_tensor("ps", [8, 1], dt.float32, kind="ExternalOutput").ap()
    pt_d = nc.dram_tensor("pt", [128, 1], dt.float32, kind="ExternalOutput").ap()
    mU_d = nc.dram_tensor("mU_s", [128, SE * C], dt.bfloat16).ap()
    mD_d = nc.dram_tensor("mD_s", [128, SE * C], dt.bfloat16).ap()
    mL_d = nc.dram_tensor("mL_s", [128, SE * C], dt.bfloat16).ap()

    with tile.TileContext(nc) as tc:
        with ExitStack() as ctx:
            big = ctx.enter_context(tc.tile_pool(name="big", bufs=1))
            st_pool = ctx.enter_context(tc.tile_pool(name="stage", bufs=2))
            win_pool = ctx.enter_context(tc.tile_pool(name="win", bufs=2))
            sc_pool = ctx.enter_context(tc.tile_pool(name="scr", bufs=2))

            h = big.tile([128, S * C], dt.float32, tag="h")
            d = big.tile([128, S * C], dt.bfloat16, tag="d")
            E = big.tile([128, SE * C], dt.float32, tag="E")
            dT = big.tile([128, S], dt.float32, tag="dT")
            dO = big.tile([128, 64], dt.float32, tag="dO")
            trow = big.tile([128, S], dt.float32, tag="trow")
            nrow = big.tile([128, 64], dt.float32, tag="nrow")
            q65 = big.tile([128, 1], dt.float32, tag="q65")
            c0 = big.tile([128, C], dt.float32, tag="c0")
            c1 = big.tile([128, C], dt.float32, tag="c1")
            G0 = big.tile([128, 66], dt.float32, tag="G0")
            G1t = big.tile([128, 66], dt.float32, tag="G1t")
            S0 = big.tile([128, 66], dt.float32, tag="S0")
            S1t = big.tile([128, 66], dt.float32, tag="S1t")
            zb = big.tile([128, 2 * C], dt.bfloat16, tag="zb")
            pt_t = big.tile([128, 1], dt.float32, tag="pt_t")
            c_tiles = [c0, c1]
            G_tiles = [G0, G1t]
            S_tiles = [S0, S1t]

            # inputs: raw rows DMA'd straight into the skewed layout
            nc.vector.memset(dT[:], 0.0)
            nc.vector.memset(dO[:], 0.0)
            for q in range(Q):
                # dT[32q+s, q+1 : q+1+N] = y_true[s, :]; dO[32q+s, :] = y_pred[s, 64q:64q+64]
                nc.sync.dma_start(out=dT[32 * q:32 * q + SPC, q + 1:q + 1 + N], in_=yt_d[:])
                nc.sync.dma_start(out=dO[32 * q:32 * q + SPC, :], in_=yp_d[:, q * 64:(q + 1) * 64])

            # omega ingredients: trow[p,t]=t ; nrow[p,m]=(m+1)+65*q(p)
            nc.gpsimd.iota(trow[:], [[1, S]], base=0, channel_multiplier=0,
                           allow_small_or_imprecise_dtypes=True)
            nc.gpsimd.iota(nrow[:], [[1, 64]], base=1, channel_multiplier=0,
                           allow_small_or_imprecise_dtypes=True)
            for q in range(Q):
                nc.gpsimd.memset(q65[32 * q:32 * q + 32, 0:1], float(65 * q))
            nc.vector.tensor_tensor(nrow[:], nrow[:],
                                    q65[:].broadcast_to([128, 64]), Alu.add)

            # E zero on gpsimd (runs concurrent with fwd on DVE)
            nc.gpsimd.memset(E[:], 0.0)
            nc.gpsimd.memset(zb[:], 0.0)

            # D build: d[p, t*C+1+jl] = (dT[p,t]-dO[p,jl])^2  (bf16)
            nc.vector.memset(d[:], 0.0)
            DCH = 33
            for k0 in range(0, S, DCH):
                k1 = min(k0 + DCH, S)
                d3 = d[:].rearrange("p (s c) -> p s c", c=C)[:, k0:k1, 1:]
                nc.vector.tensor_tensor(
                    d3, dT[:, k0:k1].unsqueeze(2).broadcast_to([128, k1 - k0, 64]),
                    dO[:].unsqueeze(1).broadcast_to([128, k1 - k0, 64]), Alu.subtract)
                nc.vector.tensor_tensor(d3, d3, d3, Alu.mult)

            # fwd prefills
            for q in range(Q):
                nc.vector.memset(h[32 * q:32 * q + 32, q * C:(q + 1) * C], BIG)
            nc.vector.memset(h[0:8, 0:1], 0.0)
            for ct in c_tiles:
                nc.vector.memset(ct[0:32, 0:1], BIG)
            for gt in G_tiles:
                nc.vector.memset(gt[:, 0:1], 0.0)
                nc.vector.memset(gt[96:128, 65:66], 0.0)

            # ---------------- forward ----------------
            def prange(qlo, qhi):
                P0, P1 = 32 * qlo, 32 * qhi + 32
                cnt = P1 - P0
                if not (cnt <= 32 or P0 == 0 or (P0 == 64 and cnt <= 64)):
                    P0 = 0
                return P0, P1

            for t in range(1, S):
                qlo, qhi = max(0, t - 256), min(3, t - 1)
                P0, P1 = prange(qlo, qhi)
                ct = c_tiles[t % 2]
                for q in range(max(1, qlo), qhi + 1):
                    nc.gpsimd.tensor_copy(
                        ct[32 * q:32 * q + 32, 0:1],
                        h[32 * (q - 1):32 * q, (t - 1) * C + 64:(t - 1) * C + 65])
                nc.vector.tensor_tensor(
                    ct[P0:P1, 1:65],
                    h[P0:P1, (t - 1) * C + 1:(t - 1) * C + 65],
                    h[P0:P1, (t - 1) * C:(t - 1) * C + 64], Alu.min)
                # state = min(c'_j, state) + d_j  (c' excludes d; chain in c'[0])
                nc.vector.tensor_tensor_scan(
                    h[P0:P1, t * C:t * C + 65],
                    ct[P0:P1, 0:65],
                    d[P0:P1, t * C:t * C + 65], float(BIG), Alu.min, Alu.add)

            # loss_shape partials
            nc.sync.dma_start(out=ps_d[:], in_=h[96:104, 259 * C + 64:259 * C + 65])

            # ---------------- mask phase ----------------
            for s0 in range(1, S, MCH):
                s1 = min(s0 + MCH, S)
                ns = s1 - s0
                cX = sc_pool.tile([128, MCH * C], dt.float32, tag="cX")
                mu = st_pool.tile([128, MCH * C], dt.bfloat16, tag="mu")
                md = st_pool.tile([128, MCH * C], dt.bfloat16, tag="md")
                ml = st_pool.tile([128, MCH * C], dt.bfloat16, tag="ml")
                hv = h[:].rearrange("p (s c) -> p s c", c=C)
                dv = d[:].rearrange("p (s c) -> p s c", c=C)
                cXv = cX[:].rearrange("p (s c) -> p s c", c=C)[:, 0:ns, :]
                for m_t, hoff in ((mu, hv[:, s0 - 1:s1 - 1, 1:]),
                                  (md, hv[:, s0 - 1:s1 - 1, 0:64]),
                                  (ml, hv[:, s0:s1, 0:64])):
                    nc.vector.tensor_tensor(cXv[:, :, 1:], dv[:, s0:s1, 1:], hoff, Alu.add)
                    mv = m_t[:].rearrange("p (s c) -> p s c", c=C)[:, 0:ns, :]
                    nc.vector.tensor_tensor(mv[:, :, 1:], hv[:, s0:s1, 1:],
                                            cXv[:, :, 1:], Alu.is_equal)
                # margins on md, ml
                for m_t in (md, ml):
                    mv = m_t[:].rearrange("p (s c) -> p s c", c=C)[:, 0:ns, :]
                    for q in (0, 1, 2):
                        nc.gpsimd.tensor_copy(
                            mv[32 * q:32 * q + 32, :, 0:1],
                            mv[32 * (q + 1):32 * (q + 1) + 32, :, 1:2])
                    nc.gpsimd.memset(mv[96:128, :, 0:1], 0.0)
                for m_t, m_dram in ((mu, mU_d), (md, mD_d), (ml, mL_d)):
                    nc.sync.dma_start(out=m_dram[0:104, s0 * C:s1 * C],
                                      in_=m_t[0:104, 0:ns * C])
            # zero-fill DRAM mask slots 260..261
            for m_dram in (mU_d, mD_d, mL_d):
                nc.sync.dma_start(out=m_dram[0:104, 260 * C:262 * C], in_=zb[0:104, :])

            # omega build over d (all mask-phase reads of d are done):
            # d[p, t*C+1+ml] = (t - (65q + ml+1)) = i - j  for the cell this
            # slot holds; squared later via the double multiply in reduction.
            for k0 in range(0, S, DCH):
                k1 = min(k0 + DCH, S)
                d3 = d[:].rearrange("p (s c) -> p s c", c=C)[:, k0:k1, 1:]
                nc.vector.tensor_tensor(
                    d3, trow[:, k0:k1].unsqueeze(2).broadcast_to([128, k1 - k0, 64]),
                    nrow[:].unsqueeze(1).broadcast_to([128, k1 - k0, 64]), Alu.subtract)
            # zero invalid slots (t outside [q+1, q+256]) and record col 0
            dv = d[:].rearrange("p (s c) -> p s c", c=C)
            nc.vector.memset(dv[:, :, 0:1], 0.0)
            for q in range(Q):
                nc.vector.memset(d[32 * q:32 * q + 32, 0:(q + 1) * C], 0.0)
                if 257 + q < S:
                    nc.vector.memset(d[32 * q:32 * q + 32, (257 + q) * C:S * C], 0.0)

            # ---------------- backward ----------------
            def win_load(k):
                w0 = k * WCH
                nsl = min(WCH + 2, SE - w0)
                tiles = {}
                for name, m_dram in (("u", mU_d), ("d", mD_d), ("l", mL_d)):
                    w = win_pool.tile([128, (WCH + 2) * C], dt.bfloat16, tag="w" + name)
                    nc.sync.dma_start(out=w[0:104, 0:nsl * C],
                                      in_=m_dram[0:104, w0 * C:(w0 + nsl) * C])
                    tiles[name] = w
                return tiles

            cur_k = (S - 1) // WCH
            wins = {cur_k: win_load(cur_k)}
            if cur_k - 1 >= 0:
                wins[cur_k - 1] = win_load(cur_k - 1)
            for t in range(S - 1, 0, -1):
                k = t // WCH
                if k != cur_k:
                    cur_k = k
                    wins.pop(k + 2, None)
                    if k - 1 >= 0 and (k - 1) not in wins:
                        wins[k - 1] = win_load(k - 1)
                W = wins[k]
                lo = (t - k * WCH) * C
                qlo, qhi = max(0, t - 256), min(3, t - 1)
                P0, P1 = prange(qlo, qhi)
                G = G_tiles[t % 2]
                Sc = S_tiles[t % 2]
                for q in (2, 1, 0):
                    nc.vector.tensor_copy(
                        G[32 * q:32 * q + 32, 65:66],
                        E[32 * (q + 1):32 * (q + 2), (t + 1) * C + 1:(t + 1) * C + 2])
                nc.vector.tensor_tensor(
                    G[P0:P1, 1:65], E[P0:P1, (t + 1) * C + 1:(t + 1) * C + 65],
                    W["u"][P0:P1, lo + C + 1:lo + C + 65], Alu.mult)
                nc.vector.tensor_tensor(
                    Sc[P0:P1, 1:65], E[P0:P1, (t + 1) * C + 2:(t + 1) * C + 66],
                    W["d"][P0:P1, lo + C + 2:lo + C + 66], Alu.mult)
                nc.vector.tensor_tensor(G[P0:P1, 1:65], G[P0:P1, 1:65],
                                        Sc[P0:P1, 1:65], Alu.add)
                if t == S - 1:
                    nc.vector.memset(G[96:128, 64:65], 1.0)
                nc.vector.tensor_tensor_scan(
                    E[P0:P1, t * C:t * C + 66][:, ::-1],
                    W["l"][P0:P1, lo + 1:lo + 67][:, ::-1],
                    G[P0:P1, 0:66][:, ::-1], 0.0, Alu.mult, Alu.add)

            # ---------------- omega reduction ----------------
            nc.vector.tensor_tensor(E[0:104, 0:S * C], E[0:104, 0:S * C],
                                    d[0:104, 0:S * C], Alu.mult)
            nc.vector.tensor_tensor(E[0:104, 0:S * C], E[0:104, 0:S * C],
                                    d[0:104, 0:S * C], Alu.mult)
            nc.vector.tensor_reduce(
                pt_t[0:104, 0:1],
                E[0:104, 0:S * C].rearrange("p (s c) -> p s c", c=C),
                mybir.AxisListType.XY, Alu.add)
            nc.sync.dma_start(out=pt_d[:], in_=pt_t[:])

    nc.compile()
    return nc


def _get_runner():
    """Build nc + the jitted shard_map dispatcher once; cache both."""
    import jax
    from jax.sharding import Mesh, PartitionSpec
    import warnings
    with warnings.catch_warnings():
        warnings.simplefilter("ignore")
        from jax.experimental.shard_map import shard_map
    from concourse import mybir
    from concourse.bass2jax import (_bass_exec_p, partition_id_tensor,
                                    install_neuronx_cc_hook)

    nc = _build()
    install_neuronx_cc_hook()

    partition_name = nc.partition_id_tensor.name if nc.partition_id_tensor else None
    in_names, out_names, out_avals, zero_shapes = [], [], [], []
    for alloc in nc.m.functions[0].allocations:
        if not isinstance(alloc, mybir.MemoryLocationSet):
            continue
        name = alloc.memorylocations[0].name
        if alloc.kind == "ExternalInput":
            if name != partition_name:
                in_names.append(name)
        elif alloc.kind == "ExternalOutput":
            out_names.append(name)
            shape = tuple(alloc.tensor_shape)
            dtype = mybir.dt.np(alloc.dtype)
            out_avals.append(jax.core.ShapedArray(shape, dtype))
            zero_shapes.append((shape, dtype))
    n_params = len(in_names)
    n_outs = len(out_names)
    in_names_all = list(in_names) + list(out_names)
    if partition_name is not None:
        in_names_all.append(partition_name)
    donate = tuple(range(n_params, n_params + n_outs))

    dbg_name = nc.dbg_addr.name if nc.dbg_addr is not None else None
    assert dbg_name is None or dbg_name in in_names

    import jax.numpy as jnp

    def _bass_outs(args):
        operands = list(args)
        if partition_name is not None:
            operands.append(partition_id_tensor())
        outs = _bass_exec_p.bind(
            *operands, out_avals=tuple(out_avals), in_names=tuple(in_names_all),
            out_names=tuple(out_names), lowering_input_output_aliases=(),
            sim_require_finite=True, sim_require_nnan=True, nc=nc)
        return dict(zip(out_names, outs))

    def _body_scalar(*args):
        o = _bass_outs(args)
        shape_sum = jnp.sum(o["ps"])                       # 8 sample losses
        temp_sum = jnp.sum(o["pt"].reshape(Q, 32)[:, :SPC])
        part = (ALPHA * shape_sum / B
                + (1.0 - ALPHA) * temp_sum / (B * N * N))
        total = jax.lax.psum(part, "core")
        return (total.reshape(1),)

    def _body_raw(*args):
        o = _bass_outs(args)
        return tuple(o[name] for name in out_names)

    devices = jax.devices()[:NCORES]
    mesh = Mesh(np.asarray(devices), ("core",))
    in_specs = (PartitionSpec("core"),) * (n_params + n_outs)
    fn_scalar = jax.jit(
        shard_map(_body_scalar, mesh=mesh, in_specs=in_specs,
                  out_specs=(PartitionSpec(),), check_rep=False),
        donate_argnums=donate, keep_unused=True)
    fn_raw = jax.jit(
        shard_map(_body_raw, mesh=mesh, in_specs=in_specs,
                  out_specs=(PartitionSpec("core"),) * n_outs, check_rep=False),
        donate_argnums=donate, keep_unused=True)
    return {"fn": fn_scalar, "fn_raw": fn_raw, "in_names": in_names,
            "out_names": out_names, "zero_shapes": zero_shapes}


def kernel(y_pred, y_true):
    yp = np.ascontiguousarray(np.asarray(y_pred, dtype=f32).reshape(B, N))
    yt = np.ascontiguousarray(np.asarray(y_true, dtype=f32).reshape(B, N))
    if "runner" not in _cache:
        _cache["runner"] = _get_runner()
    r = _cache["runner"]

    # concat per-core inputs along axis 0: core c gets rows [c*SPC, (c+1)*SPC)
    feed = {"yt": yt, "yp": yp}
    concat_in = [feed[name] for name in r["in_names"]]
    concat_zeros = [np.zeros((NCORES * s[0], *s[1:]), dt)
                    for s, dt in r["zero_shapes"]]
    import jax
    if not r.get("scalar_bad"):
        try:
            out = r["fn"](*concat_in, *concat_zeros)
            loss = jax.device_get(out[0]).reshape(-1)[0]
            return np.array(loss, dtype=f32)
        except Exception:
            r["scalar_bad"] = True
            concat_zeros = [np.zeros((NCORES * s[0], *s[1:]), dt)
                            for s, dt in r["zero_shapes"]]

    out = r["fn_raw"](*concat_in, *concat_zeros)
    outs = dict(zip(r["out_names"], jax.device_get(out)))
    ps = outs["ps"].reshape(NCORES, 8)          # per-core loss_shape partials
    pt = outs["pt"].reshape(NCORES, Q, 32)      # per-lane omega partial sums

    loss_shape = float(np.mean(ps))
    temp_sum = float(np.sum(pt[:, :, :SPC]))
    loss_temporal = temp_sum / B / (N * N)
    loss = ALPHA * loss_shape + (1.0 - ALPHA) * loss_temporal
    return np.array(loss, dtype=f32)
